# revision 1
# baseline (speedup 1.0000x reference)
"""Trainium2 Bass kernel for nn_Conv_27693949125154.

Each 128-dim vector is a 16x8 image; valid 3x3 conv with the fixed kernel
[[1,0,1],[0,1,0],[1,0,1]] then relu -> 84 outputs (14x6).

The conv kernel decomposes as outer([1,0,1],[1,0,1]) + center tap:
    h(i,j) = x(i,j) + x(i,j+2)            (horizontal, 16x6)
    out(i,j) = relu(h(i,j) + h(i+2,j) + x(i+1,j+1))   (14x6)

Layout: letters (B*W rows) on SBUF partitions, the 128 pixels of each
letter along the free dim. All 5 stencil taps become free-dim strided
slices, so the whole conv is 3 DVE tensor-adds + 1 ACT relu per chunk.

DMA strategy (measured): half-core 7MiB input loads double-buffered on the
sync HWDGE ring; 1.26MiB output stores on the scalar HWDGE ring (separate
ring measurably improves mixed read/write throughput). Compute runs on
slices of the big input tile in chunks of 28 letters/partition.

Pure data parallel over 8 NeuronCores (batch sharding, no comm).
"""

import numpy as np

import concourse.bass as bass
import concourse.mybir as mybir
from concourse import tile
from concourse.bass_utils import run_bass_kernel_spmd

# Full problem: x (16384, 14, 128) f32 -> out (16384, 14, 84) f32
B, W, L = 16384, 14, 128
OUT = 84
N_CORES = 8
ROWS = B * W                     # 229376 letters total
ROWS_PER_CORE = ROWS // N_CORES  # 28672
P = 128                          # SBUF partitions

F32 = mybir.dt.float32


def split_multi_waits(nc, max_waits=1):
    """walrus CoreV3 codegen rejects instructions with several sync-wait
    conditions; hoist extras onto NOPs inserted just before, same engine."""
    for f in nc.m.functions:
        for blk in f.blocks:
            new = []
            for inst in blk.instructions:
                si = inst.sync_info
                if si is not None and si.on_wait and len(si.on_wait) > max_waits:
                    waits = list(si.on_wait)
                    head, tail = waits[:-max_waits], waits[-max_waits:]
                    for k, w in enumerate(head):
                        new.append(
                            mybir.InstNoOp(
                                name=f"{inst.name}-wsplit{k}",
                                engine=inst.engine,
                                ins=[],
                                outs=[],
                                sync_info=mybir.SyncInfo(on_wait=[w], on_update=[]),
                            )
                        )
                    inst.sync_info = mybir.SyncInfo(
                        on_wait=tail, on_update=list(si.on_update)
                    )
                new.append(inst)
            blk.instructions = new


def build_program(rows=ROWS_PER_CORE, read_sizes=None, chunk_sizes=None,
                  split_waits=True, o_bufs=2, work_bufs=2, r_bufs=2,
                  op1_engine="vector"):
    """Per-core program: x [rows,128] f32 -> y [rows,84] f32.

    The whole per-core input stays resident in SBUF (t_total*512B per
    partition). Reads are issued upfront as independent slice-DMAs
    (deep read-ahead, no buffer reuse); compute runs in letter chunks;
    relu'd outputs stream out on the scalar ring. First/last chunks are
    smaller to shorten the pipeline ramp and tail.
    """
    t_total = rows // P                  # letters per partition (224)
    if read_sizes is None:
        read_sizes = [4, 4, 6, 14, 14, 14] + [28] * ((t_total - 56) // 28)
    if chunk_sizes is None:
        chunk_sizes = [7, 14, 42, 42, 42, 42, 21, 7, 7]
    assert sum(read_sizes) == t_total and sum(chunk_sizes) == t_total
    t_c_max = max(chunk_sizes)

    nc = bass.Bass(
        "TRN2", target_bir_lowering=False, debug=False, num_devices=N_CORES
    )
    x = nc.dram_tensor("x", [rows, L], F32, kind="ExternalInput")
    y = nc.dram_tensor("y", [rows, OUT], F32, kind="ExternalOutput")

    # partition p holds letters [p*t_total, (p+1)*t_total)
    xf = x.ap().rearrange("(p t) m -> p (t m)", p=P)   # [P, t_total*128]
    yf = y.ap().rearrange("(p t) m -> p (t m)", p=P)   # [P, t_total*84]

    with tile.TileContext(nc) as tc:
        with (
            tc.tile_pool(name="xin", bufs=1) as xin_pool,
            tc.tile_pool(name="oout", bufs=o_bufs) as oout_pool,
            tc.tile_pool(name="work", bufs=work_bufs) as work,
            tc.tile_pool(name="rpool", bufs=r_bufs) as rpool,
        ):
            xt = xin_pool.tile([P, t_total * L], F32, tag="x")
            # all reads upfront into disjoint slices -> max read-ahead
            off = 0
            for k, sz in enumerate(read_sizes):
                eng = nc.scalar if (k % 2 == 1 and k < 8) else nc.sync
                eng.dma_start(
                    out=xt[:, off * L : (off + sz) * L],
                    in_=xf[:, off * L : (off + sz) * L],
                )
                off += sz

            X3 = xt.rearrange("p (row c) -> p row c", c=8)       # [P,t*16,8]
            X4 = xt.rearrange("p (t i j) -> p t i j", i=16, j=8)  # [P,t,16,8]
            off = 0
            for t_c in chunk_sizes:
                # h(i,j) = x(i,j) + x(i,j+2) over t_c*16 rows
                r = rpool.tile([P, t_c_max * 96], F32, tag="r", name="r")[:, : t_c * 96]
                x3 = X3[:, off * 16 : (off + t_c) * 16]         # [P, t_c*16, 8]
                r3 = r.rearrange("p (row c) -> p row c", c=6)
                op1 = nc.gpsimd if op1_engine == "gpsimd" else nc.vector
                op1.tensor_tensor(
                    r3[:], x3[:, :, 0:6], x3[:, :, 2:8], mybir.AluOpType.add
                )

                # s = h(rows 0..13) + center taps x(1..14, 1..6)
                s = work.tile([P, t_c_max * 84], F32, tag="s", name="s")[:, : t_c * 84]
                r4 = r.rearrange("p (t i j) -> p t i j", i=16, j=6)
                x4 = X4[:, off : off + t_c]                     # [P, t_c, 16, 8]
                s4 = s.rearrange("p (t i j) -> p t i j", i=14, j=6)
                nc.vector.tensor_tensor(
                    s4[:], r4[:, :, 0:14, :], x4[:, :, 1:15, 1:7],
                    mybir.AluOpType.add,
                )

                # u = s + h(rows 2..15), in place over s
                nc.vector.tensor_tensor(
                    s4[:], s4[:], r4[:, :, 2:16, :], mybir.AluOpType.add
                )

                # relu on the scalar engine; out-DMA on the scalar ring
                ot = oout_pool.tile([P, t_c_max * OUT], F32, tag="o", name="ot")[:, : t_c * OUT]
                nc.scalar.activation(
                    ot[:], s[:], mybir.ActivationFunctionType.Relu
                )
                nc.scalar.dma_start(
                    out=yf[:, off * OUT : (off + t_c) * OUT], in_=ot[:]
                )
                off += t_c

    if split_waits:
        split_multi_waits(nc)
    return nc


_nc_cache = {}


def _get_program():
    if "nc" not in _nc_cache:
        _nc_cache["nc"] = build_program()
    return _nc_cache["nc"]


def kernel(x):
    x = np.ascontiguousarray(np.asarray(x, dtype=np.float32))
    assert x.shape == (B, W, L), x.shape

    nc = _get_program()
    shards = x.reshape(N_CORES, ROWS_PER_CORE, L)
    in_maps = [{"x": shards[i]} for i in range(N_CORES)]
    res = run_bass_kernel_spmd(nc, in_maps, core_ids=list(range(N_CORES)))
    out = np.concatenate(
        [res.results[i]["y"].reshape(-1, W, OUT) for i in range(N_CORES)], axis=0
    )
    return out



# revision 2
# speedup vs baseline: 1.0225x; 1.0225x over previous
"""Trainium2 Bass kernel for nn_Conv_27693949125154.

Each 128-dim vector is a 16x8 image; valid 3x3 conv with the fixed kernel
[[1,0,1],[0,1,0],[1,0,1]] then relu -> 84 outputs (14x6).

The conv kernel decomposes as outer([1,0,1],[1,0,1]) + center tap:
    h(i,j) = x(i,j) + x(i,j+2)            (horizontal, 16x6)
    out(i,j) = relu(h(i,j) + h(i+2,j) + x(i+1,j+1))   (14x6)

Layout: letters (B*W rows) on SBUF partitions, the 128 pixels of each
letter along the free dim. All 5 stencil taps become free-dim strided
slices, so the whole conv is 3 DVE tensor-adds + 1 ACT relu per chunk.

DMA strategy (measured): half-core 7MiB input loads double-buffered on the
sync HWDGE ring; 1.26MiB output stores on the scalar HWDGE ring (separate
ring measurably improves mixed read/write throughput). Compute runs on
slices of the big input tile in chunks of 28 letters/partition.

Pure data parallel over 8 NeuronCores (batch sharding, no comm).
"""

import numpy as np

import concourse.bass as bass
import concourse.mybir as mybir
from concourse import tile
from concourse.bass_utils import run_bass_kernel_spmd

# Full problem: x (16384, 14, 128) f32 -> out (16384, 14, 84) f32
B, W, L = 16384, 14, 128
OUT = 84
N_CORES = 8
ROWS = B * W                     # 229376 letters total
ROWS_PER_CORE = ROWS // N_CORES  # 28672
P = 128                          # SBUF partitions

F32 = mybir.dt.float32


def split_multi_waits(nc, max_waits=1):
    """walrus CoreV3 codegen rejects instructions with several sync-wait
    conditions; hoist extras onto NOPs inserted just before, same engine."""
    for f in nc.m.functions:
        for blk in f.blocks:
            new = []
            for inst in blk.instructions:
                si = inst.sync_info
                if si is not None and si.on_wait and len(si.on_wait) > max_waits:
                    waits = list(si.on_wait)
                    head, tail = waits[:-max_waits], waits[-max_waits:]
                    for k, w in enumerate(head):
                        new.append(
                            mybir.InstNoOp(
                                name=f"{inst.name}-wsplit{k}",
                                engine=inst.engine,
                                ins=[],
                                outs=[],
                                sync_info=mybir.SyncInfo(on_wait=[w], on_update=[]),
                            )
                        )
                    inst.sync_info = mybir.SyncInfo(
                        on_wait=tail, on_update=list(si.on_update)
                    )
                new.append(inst)
            blk.instructions = new


def build_program(rows=ROWS_PER_CORE, read_sizes=None, chunk_sizes=None,
                  split_waits=True, o_bufs=2, work_bufs=2, r_bufs=2,
                  op1_engine="gpsimd"):
    """Per-core program: x [rows,128] f32 -> y [rows,84] f32.

    The whole per-core input stays resident in SBUF (t_total*512B per
    partition). Reads are issued upfront as independent slice-DMAs
    (deep read-ahead, no buffer reuse); compute runs in letter chunks;
    relu'd outputs stream out on the scalar ring. First/last chunks are
    smaller to shorten the pipeline ramp and tail.
    """
    t_total = rows // P                  # letters per partition (224)
    if read_sizes is None:
        read_sizes = [4, 4, 6, 14, 14, 14] + [28] * ((t_total - 56) // 28)
    if chunk_sizes is None:
        chunk_sizes = [7, 14, 42, 42, 42, 42, 21, 7, 7]
    assert sum(read_sizes) == t_total and sum(chunk_sizes) == t_total
    t_c_max = max(chunk_sizes)

    nc = bass.Bass(
        "TRN2", target_bir_lowering=False, debug=False, num_devices=N_CORES
    )
    x = nc.dram_tensor("x", [rows, L], F32, kind="ExternalInput")
    y = nc.dram_tensor("y", [rows, OUT], F32, kind="ExternalOutput")

    # partition p holds letters [p*t_total, (p+1)*t_total)
    xf = x.ap().rearrange("(p t) m -> p (t m)", p=P)   # [P, t_total*128]
    yf = y.ap().rearrange("(p t) m -> p (t m)", p=P)   # [P, t_total*84]

    with tile.TileContext(nc) as tc:
        with (
            tc.tile_pool(name="xin", bufs=1) as xin_pool,
            tc.tile_pool(name="oout", bufs=o_bufs) as oout_pool,
            tc.tile_pool(name="work", bufs=work_bufs) as work,
            tc.tile_pool(name="rpool", bufs=r_bufs) as rpool,
        ):
            xt = xin_pool.tile([P, t_total * L], F32, tag="x")
            # all reads upfront into disjoint slices -> max read-ahead
            off = 0
            for k, sz in enumerate(read_sizes):
                eng = nc.scalar if (k % 2 == 1 and k < 8) else nc.sync
                eng.dma_start(
                    out=xt[:, off * L : (off + sz) * L],
                    in_=xf[:, off * L : (off + sz) * L],
                )
                off += sz

            X3 = xt.rearrange("p (row c) -> p row c", c=8)       # [P,t*16,8]
            X4 = xt.rearrange("p (t i j) -> p t i j", i=16, j=8)  # [P,t,16,8]
            off = 0
            for t_c in chunk_sizes:
                # h(i,j) = x(i,j) + x(i,j+2) over t_c*16 rows
                r = rpool.tile([P, t_c_max * 96], F32, tag="r", name="r")[:, : t_c * 96]
                x3 = X3[:, off * 16 : (off + t_c) * 16]         # [P, t_c*16, 8]
                r3 = r.rearrange("p (row c) -> p row c", c=6)
                op1 = nc.gpsimd if op1_engine == "gpsimd" else nc.vector
                op1.tensor_tensor(
                    r3[:], x3[:, :, 0:6], x3[:, :, 2:8], mybir.AluOpType.add
                )

                # s = h(rows 0..13) + center taps x(1..14, 1..6)
                s = work.tile([P, t_c_max * 84], F32, tag="s", name="s")[:, : t_c * 84]
                r4 = r.rearrange("p (t i j) -> p t i j", i=16, j=6)
                x4 = X4[:, off : off + t_c]                     # [P, t_c, 16, 8]
                s4 = s.rearrange("p (t i j) -> p t i j", i=14, j=6)
                nc.vector.tensor_tensor(
                    s4[:], r4[:, :, 0:14, :], x4[:, :, 1:15, 1:7],
                    mybir.AluOpType.add,
                )

                # u = s + h(rows 2..15), in place over s
                nc.vector.tensor_tensor(
                    s4[:], s4[:], r4[:, :, 2:16, :], mybir.AluOpType.add
                )

                # relu on the scalar engine; out-DMA on the scalar ring
                ot = oout_pool.tile([P, t_c_max * OUT], F32, tag="o", name="ot")[:, : t_c * OUT]
                nc.scalar.activation(
                    ot[:], s[:], mybir.ActivationFunctionType.Relu
                )
                nc.scalar.dma_start(
                    out=yf[:, off * OUT : (off + t_c) * OUT], in_=ot[:]
                )
                off += t_c

    if split_waits:
        split_multi_waits(nc)
    return nc


_nc_cache = {}


def _get_program():
    if "nc" not in _nc_cache:
        _nc_cache["nc"] = build_program()
    return _nc_cache["nc"]


def kernel(x):
    x = np.ascontiguousarray(np.asarray(x, dtype=np.float32))
    assert x.shape == (B, W, L), x.shape

    nc = _get_program()
    shards = x.reshape(N_CORES, ROWS_PER_CORE, L)
    in_maps = [{"x": shards[i]} for i in range(N_CORES)]
    res = run_bass_kernel_spmd(nc, in_maps, core_ids=list(range(N_CORES)))
    out = np.concatenate(
        [res.results[i]["y"].reshape(-1, W, OUT) for i in range(N_CORES)], axis=0
    )
    return out



# revision 3
# speedup vs baseline: 1.2362x; 1.2091x over previous
"""Trainium2 Bass kernel for nn_Conv_27693949125154.

Each 128-dim vector is a 16x8 image; valid 3x3 conv with the fixed kernel
[[1,0,1],[0,1,0],[1,0,1]] then relu -> 84 outputs (14x6).

The conv kernel decomposes as outer([1,0,1],[1,0,1]) + center tap:
    h(i,j) = x(i,j) + x(i,j+2)            (horizontal, 16x6)
    out(i,j) = relu(h(i,j) + h(i+2,j) + x(i+1,j+1))   (14x6)

Layout: letters (B*W rows) on SBUF partitions, the 128 pixels of each
letter along the free dim. All 5 stencil taps become free-dim strided
slices.

Compute pipeline per chunk (bf16 intermediates — tolerance is 2e-2, bf16
rounding contributes ~4e-3; halves DVE cycle count and SBUF byte traffic):
    op1  (DVE or Pool): r  = x(i,j)+x(i,j+2)      f32 -> bf16  (16x6/letter)
    cast (ACT):         ct = x(i+1,j+1)           f32 -> bf16  (14x6)
    op2  (DVE):         s  = r(0:14)+r(2:16)      bf16         (14x6)
    op3  (DVE):         u  = s + ct               bf16 contiguous
    relu (ACT):         ot = relu(u)              bf16 -> f32
    store (scalar ring)

Pure data parallel over 8 NeuronCores (batch sharding, no comm).
"""

import numpy as np

import concourse.bass as bass
import concourse.mybir as mybir
from concourse import tile
from concourse.bass_utils import run_bass_kernel_spmd

# Full problem: x (16384, 14, 128) f32 -> out (16384, 14, 84) f32
B, W, L = 16384, 14, 128
OUT = 84
N_CORES = 8
ROWS = B * W                     # 229376 letters total
ROWS_PER_CORE = ROWS // N_CORES  # 28672
P = 128                          # SBUF partitions

F32 = mybir.dt.float32
BF16 = mybir.dt.bfloat16


def split_multi_waits(nc, max_waits=1):
    """walrus CoreV3 codegen rejects instructions with several sync-wait
    conditions; hoist extras onto NOPs inserted just before, same engine."""
    for f in nc.m.functions:
        for blk in f.blocks:
            new = []
            for inst in blk.instructions:
                si = inst.sync_info
                if si is not None and si.on_wait and len(si.on_wait) > max_waits:
                    waits = list(si.on_wait)
                    head, tail = waits[:-max_waits], waits[-max_waits:]
                    for k, w in enumerate(head):
                        new.append(
                            mybir.InstNoOp(
                                name=f"{inst.name}-wsplit{k}",
                                engine=inst.engine,
                                ins=[],
                                outs=[],
                                sync_info=mybir.SyncInfo(on_wait=[w], on_update=[]),
                            )
                        )
                    inst.sync_info = mybir.SyncInfo(
                        on_wait=tail, on_update=list(si.on_update)
                    )
                new.append(inst)
            blk.instructions = new


def build_program(rows=ROWS_PER_CORE, read_sizes=None, chunk_sizes=None,
                  split_waits=True, o_bufs=2, work_bufs=2, r_bufs=2,
                  pool_op1_chunks=(3, 5)):
    """Per-core program: x [rows,128] f32 -> y [rows,84] f32.

    The whole per-core input stays resident in SBUF. Reads are issued
    upfront as independent slice-DMAs (deep read-ahead, no buffer reuse);
    compute runs in letter chunks; relu'd outputs stream out on the scalar
    ring. First/last chunks are smaller to shorten the pipeline ramp/tail.
    pool_op1_chunks: chunk indices whose op1 runs on the Pool engine
    (spreads ALU work off the DVE).
    """
    t_total = rows // P                  # letters per partition (224)
    if read_sizes is None:
        read_sizes = [4, 4, 6, 14, 14, 14] + [28] * ((t_total - 56) // 28)
    if chunk_sizes is None:
        chunk_sizes = [7, 14, 42, 42, 42, 42, 21, 7, 7]
    assert sum(read_sizes) == t_total and sum(chunk_sizes) == t_total
    t_c_max = max(chunk_sizes)

    nc = bass.Bass(
        "TRN2", target_bir_lowering=False, debug=False, num_devices=N_CORES
    )
    x = nc.dram_tensor("x", [rows, L], F32, kind="ExternalInput")
    y = nc.dram_tensor("y", [rows, OUT], F32, kind="ExternalOutput")

    # partition p holds letters [p*t_total, (p+1)*t_total)
    xf = x.ap().rearrange("(p t) m -> p (t m)", p=P)   # [P, t_total*128]
    yf = y.ap().rearrange("(p t) m -> p (t m)", p=P)   # [P, t_total*84]

    with tile.TileContext(nc) as tc:
        with (
            tc.tile_pool(name="xin", bufs=1) as xin_pool,
            tc.tile_pool(name="oout", bufs=o_bufs) as oout_pool,
            tc.tile_pool(name="work", bufs=work_bufs) as work,
            tc.tile_pool(name="rpool", bufs=r_bufs) as rpool,
            tc.tile_pool(name="ctpool", bufs=work_bufs) as ctpool,
            tc.tile_pool(name="upool", bufs=work_bufs) as upool,
        ):
            xt = xin_pool.tile([P, t_total * L], F32, tag="x")
            # all reads upfront into disjoint slices -> max read-ahead
            off = 0
            for k, sz in enumerate(read_sizes):
                eng = nc.scalar if (k % 2 == 1 and k < 8) else nc.sync
                eng.dma_start(
                    out=xt[:, off * L : (off + sz) * L],
                    in_=xf[:, off * L : (off + sz) * L],
                )
                off += sz

            X3 = xt.rearrange("p (row c) -> p row c", c=8)       # [P,t*16,8]
            X4 = xt.rearrange("p (t i j) -> p t i j", i=16, j=8)  # [P,t,16,8]
            off = 0
            for ci, t_c in enumerate(chunk_sizes):
                # h(i,j) = x(i,j) + x(i,j+2) over t_c*16 rows -> bf16
                r = rpool.tile([P, t_c_max * 96], BF16, tag="r", name="r")[:, : t_c * 96]
                x3 = X3[:, off * 16 : (off + t_c) * 16]         # [P, t_c*16, 8]
                r3 = r.rearrange("p (row c) -> p row c", c=6)
                op1 = nc.gpsimd if ci in pool_op1_chunks else nc.vector
                op1.tensor_tensor(
                    r3[:], x3[:, :, 0:6], x3[:, :, 2:8], mybir.AluOpType.add
                )

                # center taps cast f32 -> bf16 on the ACT engine
                ct = ctpool.tile([P, t_c_max * 84], BF16, tag="ct", name="ct")[:, : t_c * 84]
                x4 = X4[:, off : off + t_c]                     # [P, t_c, 16, 8]
                ct4 = ct.rearrange("p (t i j) -> p t i j", i=14, j=6)
                nc.scalar.activation(
                    ct4[:], x4[:, :, 1:15, 1:7], mybir.ActivationFunctionType.Copy
                )

                # s = h(rows 0..13) + h(rows 2..15)   (bf16, 2x DVE mode)
                s = work.tile([P, t_c_max * 84], BF16, tag="s", name="s")[:, : t_c * 84]
                r4 = r.rearrange("p (t i j) -> p t i j", i=16, j=6)
                s4 = s.rearrange("p (t i j) -> p t i j", i=14, j=6)
                nc.vector.tensor_tensor(
                    s4[:], r4[:, :, 0:14, :], r4[:, :, 2:16, :],
                    mybir.AluOpType.add,
                )

                # u = s + center taps (contiguous bf16)
                u = upool.tile([P, t_c_max * 84], BF16, tag="u", name="u")[:, : t_c * 84]
                nc.vector.tensor_tensor(
                    u[:], s[:], ct[:], mybir.AluOpType.add
                )

                # relu on the scalar engine (bf16 -> f32); out-DMA scalar ring
                ot = oout_pool.tile([P, t_c_max * OUT], F32, tag="o", name="ot")[:, : t_c * OUT]
                nc.scalar.activation(
                    ot[:], u[:], mybir.ActivationFunctionType.Relu
                )
                nc.scalar.dma_start(
                    out=yf[:, off * OUT : (off + t_c) * OUT], in_=ot[:]
                )
                off += t_c

    if split_waits:
        split_multi_waits(nc)
    return nc


_nc_cache = {}


def _get_program():
    if "nc" not in _nc_cache:
        _nc_cache["nc"] = build_program()
    return _nc_cache["nc"]


def kernel(x):
    x = np.ascontiguousarray(np.asarray(x, dtype=np.float32))
    assert x.shape == (B, W, L), x.shape

    nc = _get_program()
    shards = x.reshape(N_CORES, ROWS_PER_CORE, L)
    in_maps = [{"x": shards[i]} for i in range(N_CORES)]
    res = run_bass_kernel_spmd(nc, in_maps, core_ids=list(range(N_CORES)))
    out = np.concatenate(
        [res.results[i]["y"].reshape(-1, W, OUT) for i in range(N_CORES)], axis=0
    )
    return out
